# revision 20
# baseline (speedup 1.0000x reference)
"""GRU-D Trainium2 kernel (8-core SPMD, data-parallel over batch).

Model (reference): B=512, T=200, D=128, H=512.
Per-core: 64 batch samples, full T recurrence.

Decomposition
-------------
All h-independent terms are precomputed as large parallel matmuls (phase A):
    delta_x = min(1, exp(-(d*w_gx + b_gx)))                  [elementwise]
    xhat    = m*x + (1-m)*(delta_x*xl + (1-delta_x)*xm)      [elementwise]
    delta_h = min(1, exp(-(Wgh @ d + b_gh)))                 [D->H matmul]
    P_g     = Wgx_g @ xhat + Wgm_g @ m + b_g   for g in z,r,h
(split of W_g [H, 2D+H] into x / h / m column blocks; comb = [x, h, m]).

The serial scan (phase B) then only needs, per step:
    g  = delta_h_t * h
    z|r = sigmoid(P_zr_t + W{z,r}h @ g)
    u  = r * g
    ht = tanh(P_h_t + Whh @ u)
    h  = g + z*(ht - g)

Perf notes (vs the fp32 baseline):
  * All matmul operands are fp16: fp32 matmuls are emulated as 2 half-speed
    passes (2x LDWEIGHTS + 2x MATMUL each) and disable the compiler's fast
    weight load; fp16 runs 1 cycle/row with FWL.
  * P_r|P_h are preloaded into PSUM with one identity matmul per step
    (start=True), the W matmuls accumulate on top (start=False) -- the
    pre-activation adds leave the critical path.
  * r-gate matmuls run before z-gate matmuls so sigmoid(r) and u=r*g
    (scalar/vector) overlap the z matmuls; sigmoid(z) overlaps the h
    matmuls (its add runs on gpsimd).
  * h state, all activations, and the DRAM scratch are fp16 (PSUM stays
    fp32); elementwise runs at the 2x 16-bit DVE rate.

Everything on-device is stored feature-major: tensors [H, B_local] live as
SBUF tiles [128, 4*64] with column index = h_chunk*64 + b.  Weights are
stationary [128,128] lhsT tiles, the moving operand is the state (N=64).

Phase A streams per-step records to a DRAM scratch laid out
    rec[t] = [dh (256) | Pz (256) | Pr (256) | Ph (256)]   (cols, per partition)
which phase B consumes with one DMA per step.

Final projection (H->2) + batch norm run on host over the gathered
h_last (trivial FLOPs, needs cross-core batch statistics anyway).
"""

import functools
import sys
from collections import deque

for _p in ("/opt/trn_rl_repo",):
    if _p not in sys.path:
        sys.path.insert(0, _p)

import numpy as np

import concourse.bacc as bacc
import concourse.tile as tile
from concourse import mybir

AF = mybir.ActivationFunctionType
F32 = mybir.dt.float32
F16 = mybir.dt.float16

B, T_FULL, D, H = 512, 200, 128, 512
NCORES = 8
BL = B // NCORES          # 64 samples per core
MC = H // 128             # 4 h-chunks
REC = 4 * 256             # per-step record width (dh | Pz | Pr | Ph)
CHUNK = 512               # phase-A tb columns per chunk (= 8 steps worth)
TPC = CHUNK // BL         # timesteps per phase-A chunk (8)
BN_EPS = 1e-5

_nc_cache = {}


def build(T=T_FULL):
    """Build the single-core Bass program (SPMD: same program on all cores)."""
    assert T % TPC == 0
    TB = T * BL
    nchunk = TB // CHUNK

    nc = bacc.Bacc("TRN2", target_bir_lowering=False, debug=False)

    # --- external inputs (feature-major, host-prepared) ---
    def din(name, shape, dt=F16):
        return nc.dram_tensor(name, shape, dt, kind="ExternalInput")

    x_d = din("x", [128, TB])
    xl_d = din("xl", [128, TB])
    m_d = din("m", [128, TB])
    dt_d = din("dt", [128, TB])
    xm_d = din("xm", [128, TB])

    wgx_d = din("wgx_n", [128, 1], F32)      # -w_gx
    bgx_d = din("bgx_n", [128, 1], F32)      # -b_gx
    wgh_d = din("wgh_t", [128, H])           # Wgh.T
    bgh_d = din("bgh_n", [128, MC], F32)     # -b_gh  (col = h chunk)

    wxs_d = din("wx_t", [128, 3 * H])   # [Wzx.T | Wrx.T | Whx.T]
    wms_d = din("wm_t", [128, 3 * H])   # [Wzm.T | Wrm.T | Whm.T]
    whh_d = din("wh_t", [128, 3 * MC * H])  # z|r|h hidden blocks, tile (k,m) at
    #                                         g*2048 + k*512 + m*128
    bia_d = din("bias", [128, 3 * MC], F32)  # b_z | b_r | b_h  (col = g*4 + chunk)
    bi1_d = din("bias1", [1, 3 * MC * 128])  # same biases, single-partition
    idn_d = din("idn", [128, 128])           # fp16 identity

    h_out = nc.dram_tensor("h_out", [128, MC * BL], F16, kind="ExternalOutput")

    W = MC * BL  # 256

    with tile.TileContext(nc) as tc:
        with (
            tc.tile_pool(name="wsb", bufs=1) as wp,
            tc.tile_pool(name="state", bufs=1) as stp,
        ):
            # resident weights
            wgx = wp.tile([128, 1], F32, tag="wgx")
            bgx = wp.tile([128, 1], F32, tag="bgx")
            wgh = wp.tile([128, H], F16, tag="wgh")
            bgh = wp.tile([128, MC], F32, tag="bgh")
            wxs = wp.tile([128, 3 * H], F16, tag="wxs")
            wms = wp.tile([128, 3 * H], F16, tag="wms")
            whh = wp.tile([128, 3 * MC * H], F16, tag="whh")
            bia = wp.tile([128, 3 * MC], F32, tag="bia")
            bi1 = wp.tile([1, 3 * MC * 128], F16, tag="bi1")
            idn = wp.tile([128, 128], F16, tag="idn")
            one1 = wp.tile([1, CHUNK], F16, tag="one1")
            for sb_t, dr in [
                (wgx, wgx_d), (bgx, bgx_d), (wgh, wgh_d), (bgh, bgh_d),
                (wxs, wxs_d), (wms, wms_d), (whh, whh_d), (bia, bia_d),
                (bi1, bi1_d), (idn, idn_d),
            ]:
                nc.sync.dma_start(sb_t[:], dr[:])
            nc.vector.memset(one1[:], 1.0)

            h = stp.tile([128, W], F16, tag="h")
            nc.vector.memset(h[:], 0.0)

            # ---------- fused phase A + phase B (serial scan) ----------
            # Phase A work for chunk ci (8 steps of records, 28 matmuls +
            # elementwise) is queued as small items and drained ~4 matmuls
            # per scan step, two chunks ahead of consumption, filling the
            # tensor-engine gaps left by the serial chain.  Records live in
            # SBUF ring tiles (no DRAM round trip).
            #
            # Scan state is g_t = dh_t * h_{t-1} (h only materialized at the
            # last step):
            #   g' = dh' * ((1-z)g + z*tanh(Ph + Wh@(r*g)))
            #      = p + q*hts,   p = (1-z)*(dh'*g),  q = dh'*z
            HB = W // 2  # 128: half of the state columns
            with (
                tc.tile_pool(name="pin", bufs=2) as pin,
                tc.tile_pool(name="pwA", bufs=2) as pwA,
                tc.tile_pool(name="prcA", bufs=3) as prcA,
                tc.tile_pool(name="pgB", bufs=2) as pgB,
                tc.tile_pool(name="pwB", bufs=2) as pwB,
                tc.tile_pool(name="psA", bufs=2, space="PSUM") as psA,
                tc.tile_pool(name="psrh", bufs=3, space="PSUM") as psrh,
                tc.tile_pool(name="psz", bufs=2, space="PSUM") as psz,
            ):
                chunk_rc = {}
                aq = deque()  # (n_matmuls, emit_closure)

                def enqueue_chunk(ci):
                    s = ci * CHUNK
                    xt = pin.tile([128, CHUNK], F16, tag="x")
                    xlt = pin.tile([128, CHUNK], F16, tag="xl")
                    mt = pin.tile([128, CHUNK], F16, tag="m")
                    dtt = pin.tile([128, CHUNK], F16, tag="d")
                    xmt = pin.tile([128, CHUNK], F16, tag="xm")
                    rc = prcA.tile([128, TPC * REC], F16, tag="rc")
                    chunk_rc[ci] = rc

                    def rc3():
                        return rc[:].rearrange("p (t c) -> p t c", c=REC)

                    def it_dma():
                        nc.sync.dma_start(xt[:], x_d[:, s:s + CHUNK])
                        nc.sync.dma_start(xlt[:], xl_d[:, s:s + CHUNK])
                        nc.sync.dma_start(mt[:], m_d[:, s:s + CHUNK])
                        nc.sync.dma_start(dtt[:], dt_d[:, s:s + CHUNK])
                        nc.sync.dma_start(xmt[:], xm_d[:, s:s + CHUNK])
                    aq.append((0, it_dma))

                    def it_dh(mi):
                        pdm = psA.tile([128, CHUNK], F32, tag="psA")
                        nc.tensor.matmul(
                            pdm[:], wgh[:, mi * 128:(mi + 1) * 128], dtt[:],
                            start=True, stop=True,
                        )
                        nc.scalar.activation(
                            rc3()[:, :, mi * BL:(mi + 1) * BL],
                            pdm[:].rearrange("p (t b) -> p t b", b=BL),
                            AF.Exp, bias=bgh[:, mi:mi + 1], scale=-1.0,
                        )
                    for mi in range(MC):
                        aq.append((1, functools.partial(it_dh, mi)))

                    def it_min():
                        dh_v = rc3()[:, :, 0:256]
                        nc.vector.tensor_scalar_min(dh_v, dh_v, 1.0)
                    aq.append((0, it_min))

                    dxe = pwA.tile([128, CHUNK], F16, tag="dxe")
                    dx = pwA.tile([128, CHUNK], F16, tag="dx")
                    t2b = pwA.tile([128, CHUNK], F16, tag="t2b")
                    xh = pwA.tile([128, CHUNK], F16, tag="xh")

                    def it_dx():
                        nc.scalar.activation(
                            dxe[:], dtt[:], AF.Exp,
                            bias=bgx[:, 0:1], scale=wgx[:, 0:1],
                        )
                        nc.vector.tensor_scalar_min(dx[:], dxe[:], 1.0)
                    aq.append((0, it_dx))

                    def it_x1():
                        # t2b = dx*(xl - xm) + xm  (reuse dxe as scratch)
                        nc.vector.tensor_sub(dxe[:], xlt[:], xmt[:])
                        nc.vector.tensor_mul(dx[:], dx[:], dxe[:])
                        nc.vector.tensor_add(t2b[:], dx[:], xmt[:])
                    aq.append((0, it_x1))

                    def it_x2():
                        # xh = m*(x - t2b) + t2b
                        nc.vector.tensor_sub(dxe[:], xt[:], t2b[:])
                        nc.vector.tensor_mul(dxe[:], mt[:], dxe[:])
                        nc.vector.tensor_add(xh[:], dxe[:], t2b[:])
                    aq.append((0, it_x2))

                    def it_pg(gi, mi):
                        pp = psA.tile([128, CHUNK], F32, tag="psA")
                        wcol = gi * H + mi * 128
                        nc.tensor.matmul(
                            pp[:], wxs[:, wcol:wcol + 128], xh[:],
                            start=True, stop=False,
                        )
                        nc.tensor.matmul(
                            pp[:], wms[:, wcol:wcol + 128], mt[:],
                            start=False, stop=False,
                        )
                        # bias via K=1 matmul (bias col x ones row) so the
                        # psum->rec move is a plain table-free copy
                        bcol = (gi * MC + mi) * 128
                        nc.tensor.matmul(
                            pp[:], bi1[0:1, bcol:bcol + 128], one1[0:1, :],
                            start=False, stop=True,
                        )
                        dst = rc3()[:, :, 256 + gi * 256 + mi * BL:
                                    256 + gi * 256 + (mi + 1) * BL]
                        src = pp[:].rearrange("p (t b) -> p t b", b=BL)
                        if (gi * MC + mi) % 2 == 0:
                            nc.scalar.activation(dst, src, AF.Copy)
                        else:
                            nc.vector.tensor_copy(dst, src)
                    for gi in range(3):
                        for mi in range(MC):
                            aq.append((2, functools.partial(it_pg, gi, mi)))

                def drain(budget):
                    spent = 0
                    while aq:
                        n, fn = aq[0]
                        if spent + n > budget:
                            break
                        aq.popleft()
                        fn()
                        spent += n

                # prologue: first two chunks computed up front
                for ci in range(min(2, nchunk)):
                    enqueue_chunk(ci)
                    drain(1 << 30)

                g = pgB.tile([128, W], F16, tag="g")
                nc.vector.memset(g[:], 0.0)  # g(0) = dh*0

                for t in range(T):
                    ci0, row = divmod(t, TPC)
                    if row == 0 and ci0 + 2 < nchunk:
                        enqueue_chunk(ci0 + 2)
                    rc = chunk_rc[ci0]
                    o = row * REC
                    cin, rown = divmod(t + 1, TPC)
                    last_step = t + 1 >= T
                    if not last_step:
                        rcn = chunk_rc[cin]
                        on = rown * REC

                    # preload Pr|Ph into one psum bank (identity matmul);
                    # depends only on rec -> runs during prev step's tail
                    prh = psrh.tile([128, 512], F32, tag="prh")
                    nc.tensor.matmul(
                        prh[:], idn[:], rc[:, o + 512:o + 1024],
                        start=True, stop=False, skip_group_check=True,
                    )

                    # w = dh' * g: ready at step start, feeds p off-path
                    if not last_step:
                        w = pwB.tile([128, W], F16, tag="w")
                        nc.vector.tensor_mul(w[:], rcn[:, on:on + W], g[:])

                    # r-gate matmuls accumulate onto Pr (cols 0:256)
                    for mi in range(MC):
                        for k in range(MC):
                            wcol = 1 * MC * H + k * H + mi * 128
                            nc.tensor.matmul(
                                prh[:, mi * BL:(mi + 1) * BL],
                                whh[:, wcol:wcol + 128],
                                g[:, k * BL:(k + 1) * BL],
                                start=False, stop=False,
                                skip_group_check=True,
                            )
                    # sigmoid(r) + u in halves so h-mms start earlier
                    r16 = pwB.tile([128, W], F16, tag="r16")
                    u = pwB.tile([128, W], F16, tag="u")
                    for hf in range(2):
                        sl = slice(hf * HB, (hf + 1) * HB)
                        nc.scalar.activation(r16[:, sl], prh[:, sl], AF.Sigmoid)
                        nc.vector.tensor_mul(u[:, sl], r16[:, sl], g[:, sl])

                    # z-gate matmuls (own bank) overlap sigmoid(r)/u
                    pz = psz.tile([128, 512], F32, tag="pz")
                    for mi in range(MC):
                        for k in range(MC):
                            wcol = 0 * MC * H + k * H + mi * 128
                            nc.tensor.matmul(
                                pz[:, mi * BL:(mi + 1) * BL],
                                whh[:, wcol:wcol + 128],
                                g[:, k * BL:(k + 1) * BL],
                                start=(mi == 0 and k == 0), stop=False,
                                skip_group_check=True,
                            )

                    # h-tilde matmuls accumulate onto Ph (cols 256:512)
                    for mi in range(MC):
                        for k in range(MC):
                            wcol = 2 * MC * H + k * H + mi * 128
                            nc.tensor.matmul(
                                prh[:, W + mi * BL:W + (mi + 1) * BL],
                                whh[:, wcol:wcol + 128],
                                u[:, k * BL:(k + 1) * BL],
                                start=False, stop=False,
                                skip_group_check=True,
                            )

                    # z path (overlaps h matmuls): zpre -> z16;
                    # oz = 1-z, p = oz*w on vector; q = dh'*z on gpsimd
                    zpre = pwB.tile([128, W], F16, tag="zpre")
                    nc.vector.tensor_add(zpre[:], pz[:, 0:W],
                                         rc[:, o + 256:o + 512])
                    z16 = pwB.tile([128, W], F16, tag="z16")
                    nc.scalar.activation(z16[:], zpre[:], AF.Sigmoid)
                    oz = pwB.tile([128, W], F16, tag="oz")
                    nc.vector.tensor_scalar(
                        oz[:], z16[:], 1.0, -1.0,
                        mybir.AluOpType.subtract, mybir.AluOpType.mult,
                    )
                    if not last_step:
                        p = pwB.tile([128, W], F16, tag="p")
                        q = pwB.tile([128, W], F16, tag="q")
                        pq = pwB.tile([128, W], F16, tag="pq")
                        nc.vector.tensor_mul(p[:], oz[:], w[:])
                        nc.gpsimd.tensor_mul(q[:], rcn[:, on:on + W], z16[:])
                        nc.gpsimd.tensor_sub(pq[:], p[:], q[:])

                    # phase-A items fill the tensor gap behind the h-mms
                    drain(4)

                    # tail in halves on vector, tanh as sigmoid (keeps the
                    # scalar activation table set at {Sigmoid, Exp}):
                    #   tanh(x) = 2*sig(2x) - 1
                    #   g' = p + q*tanh = 2*(q*sig(2x)) + (p - q)
                    hts = pwB.tile([128, W], F16, tag="hts")
                    v = pwB.tile([128, W], F16, tag="v")
                    if not last_step:
                        gn = pgB.tile([128, W], F16, tag="g")
                        for hf in range(2):
                            sl = slice(hf * HB, (hf + 1) * HB)
                            nc.scalar.activation(
                                hts[:, sl],
                                prh[:, W + hf * HB:W + (hf + 1) * HB],
                                AF.Sigmoid, scale=2.0)
                            nc.vector.tensor_mul(
                                v[:, sl], q[:, sl], hts[:, sl])
                            nc.vector.scalar_tensor_tensor(
                                gn[:, sl], v[:, sl], 2.0, pq[:, sl],
                                mybir.AluOpType.mult, mybir.AluOpType.add)
                        g = gn
                    else:
                        # h_T = (1-z)*g + z*(2*sig(2x)-1), materialized once
                        nc.scalar.activation(hts[:], prh[:, W:2 * W],
                                             AF.Sigmoid, scale=2.0)
                        tnh = pwB.tile([128, W], F16, tag="tnh")
                        nc.vector.tensor_scalar(
                            tnh[:], hts[:], 2.0, 1.0,
                            mybir.AluOpType.mult, mybir.AluOpType.subtract,
                        )
                        ozg = pwB.tile([128, W], F16, tag="ozg")
                        nc.vector.tensor_mul(ozg[:], oz[:], g[:])
                        nc.vector.tensor_mul(v[:], z16[:], tnh[:])
                        nc.vector.tensor_add(h[:], ozg[:], v[:])

                drain(1 << 30)  # should be empty; safety
            nc.sync.dma_start(h_out[:], h[:])

    nc.compile()
    return nc


def get_nc(T=T_FULL):
    if T not in _nc_cache:
        _nc_cache[T] = build(T)
    return _nc_cache[T]


# ---------------------------------------------------------------- host prep

def _feature_major(a, Tn):
    """[BL, T, D] -> [D, T*BL] fp16 with b fastest."""
    return np.ascontiguousarray(
        a.transpose(2, 1, 0).astype(np.float16)).reshape(D, Tn * BL)


def prep_shared(W_gh, b_gh, W_z, b_z, W_r, b_r, W_h, b_h, w_gx, b_gx):
    """Weight arrays shared by all cores (host layout)."""
    def split(Wf):
        return Wf[:, :D], Wf[:, D:D + H], Wf[:, D + H:]

    Wzx, Wzh, Wzm = split(W_z)
    Wrx, Wrh, Wrm = split(W_r)
    Whx, Whh_, Whm = split(W_h)

    def hid_t(Wh):
        # Wh [H, H] -> Wh.T tiles: [128, MC*H] with tile (k,m) at k*H + m*128
        return (
            Wh.T.reshape(MC, 128, H).transpose(1, 0, 2).reshape(128, MC * H)
        )

    f32, f16 = np.float32, np.float16
    return {
        "wgx_n": np.ascontiguousarray(-w_gx[:, None], f32),
        "bgx_n": np.ascontiguousarray(-b_gx[:, None], f32),
        "wgh_t": np.ascontiguousarray(W_gh.T, f16),
        "bgh_n": np.ascontiguousarray(-b_gh.reshape(MC, 128).T, f32),
        "wx_t": np.ascontiguousarray(
            np.concatenate([Wzx.T, Wrx.T, Whx.T], axis=1), f16),
        "wm_t": np.ascontiguousarray(
            np.concatenate([Wzm.T, Wrm.T, Whm.T], axis=1), f16),
        "wh_t": np.ascontiguousarray(
            np.concatenate([hid_t(Wzh), hid_t(Wrh), hid_t(Whh_)], axis=1), f16),
        "bias": np.ascontiguousarray(
            np.concatenate(
                [b.reshape(MC, 128).T for b in (b_z, b_r, b_h)], axis=1), f32),
        "bias1": np.ascontiguousarray(
            np.concatenate([np.asarray(b).ravel() for b in (b_z, b_r, b_h)])
        ).astype(f16)[None, :],
        "idn": np.eye(128, dtype=f16),
    }


def prep_core(X, X_last_obsv, Mask, Delta, xm_fm, shared, c, Tn):
    sl = slice(c * BL, (c + 1) * BL)
    m = {
        "x": _feature_major(X[sl], Tn),
        "xl": _feature_major(X_last_obsv[sl], Tn),
        "m": _feature_major(Mask[sl], Tn),
        "dt": _feature_major(Delta[sl], Tn),
        "xm": xm_fm,
    }
    m.update(shared)
    return m


def host_finish(h_outs, W_fc, b_fc, bn_gamma, bn_beta):
    """Gather per-core h_last, project to logits, batch-norm over batch."""
    h_last = np.concatenate(
        [o.reshape(128, MC, BL).transpose(2, 1, 0).reshape(BL, H)
         for o in h_outs], axis=0)                      # [B, H]
    logits = h_last.astype(np.float32) @ W_fc.T.astype(np.float32) + b_fc
    mu = logits.mean(axis=0)
    var = logits.var(axis=0)
    out = bn_gamma * (logits - mu) / np.sqrt(var + BN_EPS) + bn_beta
    return out.astype(np.float32)


def run_cores(inputs, Tn=T_FULL, trace=False):
    from concourse.bass_utils import run_bass_kernel_spmd

    inputs = {k: np.asarray(v, dtype=np.float32) for k, v in inputs.items()}
    nc = get_nc(Tn)
    shared = prep_shared(
        inputs["W_gh"], inputs["b_gh"], inputs["W_z"], inputs["b_z"],
        inputs["W_r"], inputs["b_r"], inputs["W_h"], inputs["b_h"],
        inputs["w_gx"], inputs["b_gx"],
    )
    xm_fm = np.ascontiguousarray(
        np.broadcast_to(
            inputs["x_mean"].transpose(2, 1, 0), (D, Tn, BL)
        ).astype(np.float16).reshape(D, Tn * BL))
    in_maps = [
        prep_core(inputs["X"], inputs["X_last_obsv"], inputs["Mask"],
                  inputs["Delta"], xm_fm, shared, c, Tn)
        for c in range(NCORES)
    ]
    res = run_bass_kernel_spmd(
        nc, in_maps, list(range(NCORES)), trace=trace,
    )
    h_outs = [res.results[c]["h_out"] for c in range(NCORES)]
    out = host_finish(h_outs, inputs["W_fc"], inputs["b_fc"],
                      inputs["bn_gamma"], inputs["bn_beta"])
    return out, res


def kernel(**inputs):
    out, _ = run_cores(inputs, Tn=T_FULL, trace=False)
    return out
